# revision 25
# baseline (speedup 1.0000x reference)
"""MeshConv (gnn_message_passing) Trainium2 Bass kernel, SPMD over 8 NeuronCores.

Per edge e with neighbor rows a0,a1,b0,b1 = x[neighbors[e, 0..3]] (zero row for
negative indices) and self row x[e]:
    desc_a = [a0+a1, |a0-a1|], desc_b = [b0+b1, |b0-b1|]
    out[e] = [x[e], desc_a+desc_b, |desc_a-desc_b|] @ W.T + b

Device formulation. With P=a0+a1, Q=b0+b1, R=a0-a1, S=b0-b1 the reference is
    out = x W1^T + (P+Q) W2^T + (|R|+|S|) W3^T + |P-Q| W4^T + ||R|-|S|| W5^T + b
Using u+v = max(u,v)+min(u,v) and |u-v| = max(u,v)-min(u,v), fold every abs-of-
difference into the weights:
    chunkA = [max(P,Q), min(P,Q), max(|R|,|S|), min(|R|,|S|)]   (128 feats)
    wa     = [W2+W4; W2-W4; W3+W5; W3-W5]^T                      (K=128)
    chunkB = [x, 1]  @ [W1; b]^T                                 (K=33, bias fold)
Everything runs in fp16 (tolerance is 2e-2; fp16 end-to-end is ~1e-3): DMA
bytes halve vs f32, DVE tensor ops hit the 2x perf mode, PE matmul/transpose
run at 1 cycle/row, and transposed PSUM evacuations copy in 2x mode.

Edges are padded to 8*31*4096 and sharded contiguously across cores; within a
4096-edge block, edge (p,g) = base + 32*p + g. Neighbor rows are staged
host-side in edge order as [a0|b0|a1|b1] per group (the on-device indirect-DMA
path on this stack only sustains ~128 indices/us, far off the memory roofline);
x is staged a second time feature-major (plus a ones row) so the chunkB matmul
needs no on-device transpose and no copies. Per block the device does:
    DVE : PQ = lo+hi, RS = lo-hi, MP = max(P,Q), 3 PSUM->SBUF chunkA evacs
    Act : RaSa = |RS|, 1 chunkA evac, 4 output evacs (f32 PSUM -> fp16 SBUF)
    Pool: mP = min(P,Q), MX = max(Ra,Sa), MN = min(Ra,Sa)   (GPSIMD, SBUF only)
    PE  : 32 transposes [128e,128f] -> PSUM, 64 matmuls (K=128 + K=33 accum)
which keeps every engine just under the ~5.1us/block DMA roofline
(2913ns nbd in + 751ns xfm in + 1456ns out, fp16, all >=512B descriptors).
"""

import numpy as np

import concourse.bass as bass
import concourse.tile as tile
from concourse import bacc, mybir
from concourse.bass_utils import run_bass_kernel_spmd
from concourse.masks import make_identity

FP16 = mybir.dt.float16
F32 = mybir.dt.float32

E = 1_000_000
C = 32
OUT = 64
NCORES = 8
G = 32                  # 128-edge groups per block
EPB = 128 * G           # edges per block = 4096
NBLK = 31               # blocks per core
SHARD = NBLK * EPB      # 126976 edges per core
E_PAD = NCORES * SHARD  # 1015808


def _build():
    nc = bacc.Bacc(
        "TRN2", target_bir_lowering=False, debug=False, num_devices=NCORES
    )
    nbd = nc.dram_tensor("nbd", [NBLK, 128, G * 4 * C], FP16, kind="ExternalInput").ap()
    xfm = nc.dram_tensor("xfm", [C + 1, NBLK, G * 128], FP16, kind="ExternalInput").ap()
    wa = nc.dram_tensor("wa", [128, OUT], FP16, kind="ExternalInput").ap()
    wx = nc.dram_tensor("wx", [C + 1, OUT], FP16, kind="ExternalInput").ap()
    out = nc.dram_tensor("out", [SHARD, OUT], FP16, kind="ExternalOutput").ap()

    add = mybir.AluOpType.add
    sub = mybir.AluOpType.subtract
    vmax = mybir.AluOpType.max
    vmin = mybir.AluOpType.min
    band = mybir.AluOpType.bitwise_and
    I16 = mybir.dt.int16

    with tile.TileContext(nc) as tc:
        with (
            tc.tile_pool(name="consts", bufs=1) as consts,
            tc.tile_pool(name="nbp", bufs=5) as nbp,
            tc.tile_pool(name="xfp", bufs=5) as xfp,
            tc.tile_pool(name="pqp", bufs=4) as pqp,
            tc.tile_pool(name="cmb", bufs=4) as cmb,
            tc.tile_pool(name="cta", bufs=8) as ctap,
            tc.tile_pool(name="outsb", bufs=5) as osp,
            tc.tile_pool(name="pa", bufs=4, space="PSUM") as pap,
            tc.tile_pool(name="po", bufs=4, space="PSUM") as pop,
        ):
            ident = consts.tile([128, 128], FP16)
            make_identity(nc, ident[:])
            wa_sb = consts.tile([128, OUT], FP16)
            nc.sync.dma_start(wa_sb[:], wa[:])
            wx_sb = consts.tile([C + 1, OUT], FP16)
            nc.sync.dma_start(wx_sb[:], wx[:])

            def _emit_out(pending):
                pb, ptiles = pending
                out_sb = osp.tile([128, G, OUT], FP16)
                for ob in range(G // 8):
                    nc.scalar.copy(out_sb[:, 8 * ob : 8 * (ob + 1), :], ptiles[ob][:])
                # issue the output DMA from the Act sequencer: on SP its
                # sem-wait on out_sb would stall input DMAs, and on Pool the
                # SWDGE path burns ~1us of Pool engine time. Act wrote out_sb
                # itself, so its in-order SEQ reaches this DMA satisfied.
                nc.scalar.dma_start(
                    out[pb * EPB : (pb + 1) * EPB].rearrange("(p g) d -> p g d", p=128),
                    out_sb[:],
                )

            pending = None
            for b in range(NBLK):
                nb_t = nbp.tile([128, G * 4 * C], FP16)
                xf_t = xfp.tile([C + 1, G, 128], FP16)
                nc.sync.dma_start(
                    xf_t[:].rearrange("c g p -> c (g p)"), xfm[:, b]
                )

                nc.sync.dma_start(nb_t[:], nbd[b])

                # PQRS holds [P|Q|Ra|Sa] per group so one strided max and one
                # strided min produce the full comb = [MP|MX|mP|MN] layout.
                # Engine split tuned to the cost model: Pool (GPSIMD, 0.42
                # efficiency, no min/max opcode, no PSUM access) gets the add;
                # DVE (2x/4x fp16 modes) gets sub, abs, min, max.
                PQRS = pqp.tile([128, G, 4 * C], FP16)
                comb = cmb.tile([128, G, 4 * C], FP16)
                nbv = nb_t[:].rearrange("p (g j) -> p g j", g=G)
                lo = nbv[:, :, 0 : 2 * C]          # [a0|b0]
                hi = nbv[:, :, 2 * C : 4 * C]      # [a1|b1]
                rs = PQRS[:, :, 2 * C : 4 * C]
                nc.gpsimd.tensor_tensor(PQRS[:, :, 0 : 2 * C], lo, hi, op=add)
                nc.vector.tensor_tensor(rs, lo, hi, op=sub)
                # abs by clearing the fp16 sign bit: tensor_scalar bitwise_and
                # runs in the DVE 4x perf mode (abs_max-vs-0 is rejected by
                # the real ISA's tensor_scalar op check)
                nc.vector.tensor_scalar(
                    rs.bitcast(I16), rs.bitcast(I16), 0x7FFF, None, band
                )
                # v[..., 0, :] = [P, Ra], v[..., 1, :] = [Q, Sa]
                v = PQRS[:].rearrange("p g (u w c) -> p g u w c", u=2, w=2)
                nc.vector.tensor_tensor(
                    comb[:, :, 0 : 2 * C], v[:, :, :, 0, :], v[:, :, :, 1, :], op=vmax
                )
                nc.vector.tensor_tensor(
                    comb[:, :, 2 * C : 4 * C], v[:, :, :, 0, :], v[:, :, :, 1, :], op=vmin
                )

                # transpose chunkA per group: [128e,128f] -> [128f,128e]; 8/bank
                ca_tiles = []
                for q in range(G // 8):
                    pa_t = pap.tile([128, 1024], FP16)
                    for j in range(8):
                        g = 8 * q + j
                        nc.tensor.transpose(
                            pa_t[:, 128 * j : 128 * (j + 1)], comb[:, g, :], ident[:]
                        )
                    ca = ctap.tile([128, 1024], FP16)
                    # PSUM->SBUF evacuation split ~1.5 DVE / ~2.5 Act so both
                    # engines stay just under the per-block DMA roofline
                    if q == 0:
                        nc.vector.tensor_copy(ca[:], pa_t[:])
                    elif q == 1:
                        nc.vector.tensor_copy(ca[:, 0:512], pa_t[:, 0:512])
                        nc.scalar.copy(ca[:, 512:1024], pa_t[:, 512:1024])
                    else:
                        nc.scalar.copy(ca[:], pa_t[:])
                    ca_tiles.append(ca)

                if pending is not None:
                    _emit_out(pending)

                po_tiles = []
                for ob in range(G // 8):
                    po_t = pop.tile([128, 8, OUT], F32)
                    for k in range(8):
                        g = 8 * ob + k
                        og = po_t[:, k, :]
                        nc.tensor.matmul(
                            og,
                            lhsT=ca_tiles[g // 8][:, 128 * (g % 8) : 128 * (g % 8 + 1)],
                            rhs=wa_sb[:],
                            start=True,
                            stop=False,
                            skip_group_check=True,
                        )
                        nc.tensor.matmul(
                            og,
                            lhsT=xf_t[:, g, :],
                            rhs=wx_sb[:],
                            start=False,
                            stop=True,
                            skip_group_check=True,
                        )
                    po_tiles.append(po_t)

                # software pipelining: the output stage (PSUM->SBUF evac +
                # store) of block b is emitted during block b+1, so Act's
                # in-order stream never head-of-line blocks the next block's
                # chunkA evacuations behind out-copies that wait on matmuls
                pending = (b, po_tiles)

            _emit_out(pending)

    nc.compile()
    return nc


_NC = None


def _get_nc():
    global _NC
    if _NC is None:
        _NC = _build()
    return _NC


def _host_prep(x, neighbors, W, b):
    x = np.ascontiguousarray(np.asarray(x, dtype=np.float32))
    neighbors = np.asarray(neighbors)
    W = np.asarray(W, dtype=np.float64)
    b = np.asarray(b, dtype=np.float64)
    assert x.shape == (E, C) and neighbors.shape == (E, 4)

    xg = np.concatenate([x, np.zeros((1, C), np.float32)], axis=0).astype(np.float16)

    nb_pad = np.full((E_PAD, 4), E, dtype=np.int64)
    nb_pad[: neighbors.shape[0]] = neighbors
    nb_pad = np.where(nb_pad < 0, E, nb_pad)
    nb_pad = nb_pad[:, [0, 2, 1, 3]]            # per edge: [a0, b0, a1, b1]
    xs_pad = np.zeros((E_PAD, C), np.float16)
    xs_pad[: x.shape[0]] = x

    # W = [W1|W2|W3|W4|W5] along the 5C input features; fold abs-of-difference
    # pairs into sum/difference weights applied to (max, min) features.
    W1, W2, W3, W4, W5 = (W[:, i * C : (i + 1) * C] for i in range(5))
    # rows ordered [MP, MX, mP, MN] to match the device comb layout
    wa = np.concatenate(
        [(W2 + W4).T, (W3 + W5).T, (W2 - W4).T, (W3 - W5).T], axis=0
    ).astype(np.float16)
    wx = np.concatenate([W1.T, b[None, :]], axis=0).astype(np.float16)

    in_maps = []
    for c in range(NCORES):
        lo, hi = c * SHARD, (c + 1) * SHARD
        # edge (blk, p, g) = lo + blk*EPB + 32p + g
        nbd = xg[nb_pad[lo:hi].ravel()].reshape(NBLK, 128, G * 4 * C)
        xfm = xs_pad[lo:hi].reshape(NBLK, 128, G, C).transpose(3, 0, 2, 1)
        xfm = np.concatenate(
            [xfm, np.ones((1, NBLK, G, 128), np.float16)], axis=0
        ).reshape(C + 1, NBLK, G * 128)
        in_maps.append(
            {
                "nbd": np.ascontiguousarray(nbd),
                "xfm": np.ascontiguousarray(xfm),
                "wa": wa,
                "wx": wx,
            }
        )

    return in_maps


def kernel(x, neighbors, W, b):
    n_edges = np.asarray(neighbors).shape[0]
    nc = _get_nc()
    in_maps = _host_prep(x, neighbors, W, b)
    res = run_bass_kernel_spmd(nc, in_maps, core_ids=list(range(NCORES)))
    outs = [r["out"] for r in res.results]
    return np.concatenate(outs, axis=0)[:n_edges].astype(np.float32)


# revision 28
# speedup vs baseline: 1.0024x; 1.0024x over previous
"""MeshConv (gnn_message_passing) Trainium2 Bass kernel, SPMD over 8 NeuronCores.

Per edge e with neighbor rows a0,a1,b0,b1 = x[neighbors[e, 0..3]] (zero row for
negative indices) and self row x[e]:
    desc_a = [a0+a1, |a0-a1|], desc_b = [b0+b1, |b0-b1|]
    out[e] = [x[e], desc_a+desc_b, |desc_a-desc_b|] @ W.T + b

Device formulation. With P=a0+a1, Q=b0+b1, R=a0-a1, S=b0-b1 the reference is
    out = x W1^T + (P+Q) W2^T + (|R|+|S|) W3^T + |P-Q| W4^T + ||R|-|S|| W5^T + b
Using u+v = max(u,v)+min(u,v) and |u-v| = max(u,v)-min(u,v), fold every abs-of-
difference into the weights:
    chunkA = [max(P,Q), min(P,Q), max(|R|,|S|), min(|R|,|S|)]   (128 feats)
    wa     = [W2+W4; W2-W4; W3+W5; W3-W5]^T                      (K=128)
    chunkB = [x, 1]  @ [W1; b]^T                                 (K=33, bias fold)
Everything runs in fp16 (tolerance is 2e-2; fp16 end-to-end is ~1e-3): DMA
bytes halve vs f32, DVE tensor ops hit the 2x perf mode, PE matmul/transpose
run at 1 cycle/row, and transposed PSUM evacuations copy in 2x mode.

Edges are padded to 8*31*4096 and sharded contiguously across cores; within a
4096-edge block, edge (p,g) = base + 32*p + g. Neighbor rows are staged
host-side in edge order as [a0|b0|a1|b1] per group (the on-device indirect-DMA
path on this stack only sustains ~128 indices/us, far off the memory roofline);
x is staged a second time feature-major (plus a ones row) so the chunkB matmul
needs no on-device transpose and no copies. Per block the device does:
    DVE : PQ = lo+hi, RS = lo-hi, MP = max(P,Q), 3 PSUM->SBUF chunkA evacs
    Act : RaSa = |RS|, 1 chunkA evac, 4 output evacs (f32 PSUM -> fp16 SBUF)
    Pool: mP = min(P,Q), MX = max(Ra,Sa), MN = min(Ra,Sa)   (GPSIMD, SBUF only)
    PE  : 32 transposes [128e,128f] -> PSUM, 64 matmuls (K=128 + K=33 accum)
which keeps every engine just under the ~5.1us/block DMA roofline
(2913ns nbd in + 751ns xfm in + 1456ns out, fp16, all >=512B descriptors).
"""

import numpy as np

import concourse.bass as bass
import concourse.tile as tile
from concourse import bacc, mybir
from concourse.bass_utils import run_bass_kernel_spmd
from concourse.masks import make_identity

FP16 = mybir.dt.float16
F32 = mybir.dt.float32

E = 1_000_000
C = 32
OUT = 64
NCORES = 8
G = 32                  # 128-edge groups per block
EPB = 128 * G           # edges per block = 4096
NBLK = 31               # blocks per core
SHARD = NBLK * EPB      # 126976 edges per core
E_PAD = NCORES * SHARD  # 1015808


def _build():
    nc = bacc.Bacc(
        "TRN2", target_bir_lowering=False, debug=False, num_devices=NCORES
    )
    nbd = nc.dram_tensor("nbd", [NBLK, 128, G * 4 * C], FP16, kind="ExternalInput").ap()
    xfm = nc.dram_tensor("xfm", [C + 1, NBLK, G * 128], FP16, kind="ExternalInput").ap()
    wa = nc.dram_tensor("wa", [128, OUT], FP16, kind="ExternalInput").ap()
    wx = nc.dram_tensor("wx", [C + 1, OUT], FP16, kind="ExternalInput").ap()
    out = nc.dram_tensor("out", [SHARD, OUT], FP16, kind="ExternalOutput").ap()

    add = mybir.AluOpType.add
    sub = mybir.AluOpType.subtract
    vmax = mybir.AluOpType.max
    vmin = mybir.AluOpType.min
    band = mybir.AluOpType.bitwise_and
    I16 = mybir.dt.int16

    with tile.TileContext(nc) as tc:
        with (
            tc.tile_pool(name="consts", bufs=1) as consts,
            tc.tile_pool(name="nbp", bufs=4) as nbp,
            tc.tile_pool(name="xfp", bufs=4) as xfp,
            tc.tile_pool(name="pqp", bufs=3) as pqp,
            tc.tile_pool(name="cmb", bufs=3) as cmb,
            tc.tile_pool(name="cta", bufs=8) as ctap,
            tc.tile_pool(name="outsb", bufs=4) as osp,
            tc.tile_pool(name="pa", bufs=4, space="PSUM") as pap,
            tc.tile_pool(name="po", bufs=4, space="PSUM") as pop,
        ):
            ident = consts.tile([128, 128], FP16)
            make_identity(nc, ident[:])
            wa_sb = consts.tile([128, OUT], FP16)
            nc.sync.dma_start(wa_sb[:], wa[:])
            wx_sb = consts.tile([C + 1, OUT], FP16)
            nc.sync.dma_start(wx_sb[:], wx[:])

            for b in range(NBLK):
                nb_t = nbp.tile([128, G * 4 * C], FP16)
                xf_t = xfp.tile([C + 1, G, 128], FP16)
                nc.sync.dma_start(
                    xf_t[:].rearrange("c g p -> c (g p)"), xfm[:, b]
                )

                nc.sync.dma_start(nb_t[:], nbd[b])

                # PQRS holds [P|Q|Ra|Sa] per group so one strided max and one
                # strided min produce the full comb = [MP|MX|mP|MN] layout.
                # Engine split tuned to the cost model: Pool (GPSIMD, 0.42
                # efficiency, no min/max opcode, no PSUM access) gets the add;
                # DVE (2x/4x fp16 modes) gets sub, abs, min, max.
                PQRS = pqp.tile([128, G, 4 * C], FP16)
                comb = cmb.tile([128, G, 4 * C], FP16)
                nbv = nb_t[:].rearrange("p (g j) -> p g j", g=G)
                lo = nbv[:, :, 0 : 2 * C]          # [a0|b0]
                hi = nbv[:, :, 2 * C : 4 * C]      # [a1|b1]
                rs = PQRS[:, :, 2 * C : 4 * C]
                nc.gpsimd.tensor_tensor(PQRS[:, :, 0 : 2 * C], lo, hi, op=add)
                nc.vector.tensor_tensor(rs, lo, hi, op=sub)
                # abs by clearing the fp16 sign bit: tensor_scalar bitwise_and
                # runs in the DVE 4x perf mode (abs_max-vs-0 is rejected by
                # the real ISA's tensor_scalar op check)
                nc.vector.tensor_scalar(
                    rs.bitcast(I16), rs.bitcast(I16), 0x7FFF, None, band
                )
                # v[..., 0, :] = [P, Ra], v[..., 1, :] = [Q, Sa]
                v = PQRS[:].rearrange("p g (u w c) -> p g u w c", u=2, w=2)
                nc.vector.tensor_tensor(
                    comb[:, :, 0 : 2 * C], v[:, :, :, 0, :], v[:, :, :, 1, :], op=vmax
                )
                nc.vector.tensor_tensor(
                    comb[:, :, 2 * C : 4 * C], v[:, :, :, 0, :], v[:, :, :, 1, :], op=vmin
                )

                # transpose chunkA per group: [128e,128f] -> [128f,128e]; 8/bank
                ca_tiles = []
                for q in range(G // 8):
                    pa_t = pap.tile([128, 1024], FP16)
                    for j in range(8):
                        g = 8 * q + j
                        nc.tensor.transpose(
                            pa_t[:, 128 * j : 128 * (j + 1)], comb[:, g, :], ident[:]
                        )
                    ca = ctap.tile([128, 1024], FP16)
                    # PSUM->SBUF evacuation split ~1.5 DVE / ~2.5 Act so both
                    # engines stay just under the per-block DMA roofline
                    if q == 0:
                        nc.vector.tensor_copy(ca[:], pa_t[:])
                    elif q == 1:
                        nc.vector.tensor_copy(ca[:, 0:512], pa_t[:, 0:512])
                        nc.scalar.copy(ca[:, 512:1024], pa_t[:, 512:1024])
                    else:
                        nc.scalar.copy(ca[:], pa_t[:])
                    ca_tiles.append(ca)

                out_sb = osp.tile([128, G, OUT], FP16)
                for ob in range(G // 8):
                    po_t = pop.tile([128, 8, OUT], F32)
                    for k in range(8):
                        g = 8 * ob + k
                        og = po_t[:, k, :]
                        nc.tensor.matmul(
                            og,
                            lhsT=ca_tiles[g // 8][:, 128 * (g % 8) : 128 * (g % 8 + 1)],
                            rhs=wa_sb[:],
                            start=True,
                            stop=False,
                            skip_group_check=True,
                        )
                        nc.tensor.matmul(
                            og,
                            lhsT=xf_t[:, g, :],
                            rhs=wx_sb[:],
                            start=False,
                            stop=True,
                            skip_group_check=True,
                        )
                    nc.scalar.copy(out_sb[:, 8 * ob : 8 * (ob + 1), :], po_t[:])

                # issue the output DMA from the Act sequencer: on SP its
                # sem-wait on out_sb would stall the next block's input DMAs,
                # and on Pool the SWDGE path burns ~1us of Pool engine time.
                # Act wrote out_sb itself, so its in-order SEQ reaches this
                # DMA with the wait already satisfied.
                nc.scalar.dma_start(
                    out[b * EPB : (b + 1) * EPB].rearrange("(p g) d -> p g d", p=128),
                    out_sb[:],
                )

    nc.compile()
    return nc


_NC = None


def _get_nc():
    global _NC
    if _NC is None:
        _NC = _build()
    return _NC


def _host_prep(x, neighbors, W, b):
    x = np.ascontiguousarray(np.asarray(x, dtype=np.float32))
    neighbors = np.asarray(neighbors)
    W = np.asarray(W, dtype=np.float64)
    b = np.asarray(b, dtype=np.float64)
    assert x.shape == (E, C) and neighbors.shape == (E, 4)

    xg = np.concatenate([x, np.zeros((1, C), np.float32)], axis=0).astype(np.float16)

    nb_pad = np.full((E_PAD, 4), E, dtype=np.int64)
    nb_pad[: neighbors.shape[0]] = neighbors
    nb_pad = np.where(nb_pad < 0, E, nb_pad)
    nb_pad = nb_pad[:, [0, 2, 1, 3]]            # per edge: [a0, b0, a1, b1]
    xs_pad = np.zeros((E_PAD, C), np.float16)
    xs_pad[: x.shape[0]] = x

    # W = [W1|W2|W3|W4|W5] along the 5C input features; fold abs-of-difference
    # pairs into sum/difference weights applied to (max, min) features.
    W1, W2, W3, W4, W5 = (W[:, i * C : (i + 1) * C] for i in range(5))
    # rows ordered [MP, MX, mP, MN] to match the device comb layout
    wa = np.concatenate(
        [(W2 + W4).T, (W3 + W5).T, (W2 - W4).T, (W3 - W5).T], axis=0
    ).astype(np.float16)
    wx = np.concatenate([W1.T, b[None, :]], axis=0).astype(np.float16)

    in_maps = []
    for c in range(NCORES):
        lo, hi = c * SHARD, (c + 1) * SHARD
        # edge (blk, p, g) = lo + blk*EPB + 32p + g
        nbd = xg[nb_pad[lo:hi].ravel()].reshape(NBLK, 128, G * 4 * C)
        xfm = xs_pad[lo:hi].reshape(NBLK, 128, G, C).transpose(3, 0, 2, 1)
        xfm = np.concatenate(
            [xfm, np.ones((1, NBLK, G, 128), np.float16)], axis=0
        ).reshape(C + 1, NBLK, G * 128)
        in_maps.append(
            {
                "nbd": np.ascontiguousarray(nbd),
                "xfm": np.ascontiguousarray(xfm),
                "wa": wa,
                "wx": wx,
            }
        )

    return in_maps


def kernel(x, neighbors, W, b):
    n_edges = np.asarray(neighbors).shape[0]
    nc = _get_nc()
    in_maps = _host_prep(x, neighbors, W, b)
    res = run_bass_kernel_spmd(nc, in_maps, core_ids=list(range(NCORES)))
    outs = [r["out"] for r in res.results]
    return np.concatenate(outs, axis=0)[:n_edges].astype(np.float32)


# revision 29
# speedup vs baseline: 1.0458x; 1.0433x over previous
"""MeshConv (gnn_message_passing) Trainium2 Bass kernel, SPMD over 8 NeuronCores.

Per edge e with neighbor rows a0,a1,b0,b1 = x[neighbors[e, 0..3]] (zero row for
negative indices) and self row x[e]:
    desc_a = [a0+a1, |a0-a1|], desc_b = [b0+b1, |b0-b1|]
    out[e] = [x[e], desc_a+desc_b, |desc_a-desc_b|] @ W.T + b

Device formulation. With P=a0+a1, Q=b0+b1, R=a0-a1, S=b0-b1 the reference is
    out = x W1^T + (P+Q) W2^T + (|R|+|S|) W3^T + |P-Q| W4^T + ||R|-|S|| W5^T + b
Using u+v = max(u,v)+min(u,v) and |u-v| = max(u,v)-min(u,v), fold every abs-of-
difference into the weights:
    chunkA = [max(P,Q), min(P,Q), max(|R|,|S|), min(|R|,|S|)]   (128 feats)
    wa     = [W2+W4; W2-W4; W3+W5; W3-W5]^T                      (K=128)
    chunkB = [x, 1]  @ [W1; b]^T                                 (K=33, bias fold)
Everything runs in fp16 (tolerance is 2e-2; fp16 end-to-end is ~1e-3): DMA
bytes halve vs f32, DVE tensor ops hit the 2x perf mode, PE matmul/transpose
run at 1 cycle/row, and transposed PSUM evacuations copy in 2x mode.

Edges are padded to 8*31*4096 and sharded contiguously across cores; within a
4096-edge block, edge (p,g) = base + 32*p + g. Neighbor rows are staged
host-side in edge order as [a0|b0|a1|b1] per group (the on-device indirect-DMA
path on this stack only sustains ~128 indices/us, far off the memory roofline);
x is staged a second time feature-major (plus a ones row) so the chunkB matmul
needs no on-device transpose and no copies. Per block the device does:
    DVE : PQ = lo+hi, RS = lo-hi, MP = max(P,Q), 3 PSUM->SBUF chunkA evacs
    Act : RaSa = |RS|, 1 chunkA evac, 4 output evacs (f32 PSUM -> fp16 SBUF)
    Pool: mP = min(P,Q), MX = max(Ra,Sa), MN = min(Ra,Sa)   (GPSIMD, SBUF only)
    PE  : 32 transposes [128e,128f] -> PSUM, 64 matmuls (K=128 + K=33 accum)
which keeps every engine just under the ~5.1us/block DMA roofline
(2913ns nbd in + 751ns xfm in + 1456ns out, fp16, all >=512B descriptors).
"""

import numpy as np

import concourse.bass as bass
import concourse.tile as tile
from concourse import bacc, mybir
from concourse.bass_utils import run_bass_kernel_spmd
from concourse.masks import make_identity

FP16 = mybir.dt.float16
F32 = mybir.dt.float32

E = 1_000_000
C = 32
OUT = 64
NCORES = 8
G = 32                  # 128-edge groups per block
EPB = 128 * G           # edges per block = 4096
NBLK = 31               # blocks per core
SHARD = NBLK * EPB      # 126976 edges per core
E_PAD = NCORES * SHARD  # 1015808


def _build():
    nc = bacc.Bacc(
        "TRN2", target_bir_lowering=False, debug=False, num_devices=NCORES
    )
    nbd = nc.dram_tensor("nbd", [NBLK, 128, G * 4 * C], FP16, kind="ExternalInput").ap()
    xfm = nc.dram_tensor("xfm", [C + 1, NBLK, G * 128], FP16, kind="ExternalInput").ap()
    wa = nc.dram_tensor("wa", [128, OUT], FP16, kind="ExternalInput").ap()
    wx = nc.dram_tensor("wx", [C + 1, OUT], FP16, kind="ExternalInput").ap()
    out = nc.dram_tensor("out", [SHARD, OUT], FP16, kind="ExternalOutput").ap()

    add = mybir.AluOpType.add
    sub = mybir.AluOpType.subtract
    vmax = mybir.AluOpType.max
    vmin = mybir.AluOpType.min
    band = mybir.AluOpType.bitwise_and
    I16 = mybir.dt.int16

    with tile.TileContext(nc) as tc:
        with (
            tc.tile_pool(name="consts", bufs=1) as consts,
            tc.tile_pool(name="nbp", bufs=4) as nbp,
            tc.tile_pool(name="xfp", bufs=4) as xfp,
            tc.tile_pool(name="pqp", bufs=3) as pqp,
            tc.tile_pool(name="cmb", bufs=3) as cmb,
            tc.tile_pool(name="cta", bufs=8) as ctap,
            tc.tile_pool(name="outsb", bufs=4) as osp,
            tc.tile_pool(name="pa", bufs=4, space="PSUM") as pap,
            tc.tile_pool(name="po", bufs=4, space="PSUM") as pop,
        ):
            ident = consts.tile([128, 128], FP16)
            make_identity(nc, ident[:])
            wa_sb = consts.tile([128, OUT], FP16)
            nc.sync.dma_start(wa_sb[:], wa[:])
            wx_sb = consts.tile([C + 1, OUT], FP16)
            nc.sync.dma_start(wx_sb[:], wx[:])

            for b in range(NBLK):
                nb_t = nbp.tile([128, G * 4 * C], FP16)
                nc.sync.dma_start(nb_t[:], nbd[b])
                xf_t = xfp.tile([C + 1, G, 128], FP16)
                nc.sync.dma_start(
                    xf_t[:].rearrange("c g p -> c (g p)"), xfm[:, b]
                )

                # PQRS holds [P|Q|Ra|Sa] per group so one strided max and one
                # strided min produce the full comb = [MP|MX|mP|MN] layout.
                # Engine split tuned to the cost model: Pool (GPSIMD, 0.42
                # efficiency, no min/max opcode, no PSUM access) gets the add;
                # DVE (2x/4x fp16 modes) gets sub, abs, min, max.
                PQRS = pqp.tile([128, G, 4 * C], FP16)
                comb = cmb.tile([128, G, 4 * C], FP16)
                nbv = nb_t[:].rearrange("p (g j) -> p g j", g=G)
                lo = nbv[:, :, 0 : 2 * C]          # [a0|b0]
                hi = nbv[:, :, 2 * C : 4 * C]      # [a1|b1]
                rs = PQRS[:, :, 2 * C : 4 * C]
                nc.gpsimd.tensor_tensor(PQRS[:, :, 0 : 2 * C], lo, hi, op=add)
                nc.vector.tensor_tensor(rs, lo, hi, op=sub)
                # abs by clearing the fp16 sign bit: tensor_scalar bitwise_and
                # runs in the DVE 4x perf mode (abs_max-vs-0 is rejected by
                # the real ISA's tensor_scalar op check)
                nc.vector.tensor_scalar(
                    rs.bitcast(I16), rs.bitcast(I16), 0x7FFF, None, band
                )
                # v[..., 0, :] = [P, Ra], v[..., 1, :] = [Q, Sa]
                v = PQRS[:].rearrange("p g (u w c) -> p g u w c", u=2, w=2)
                nc.vector.tensor_tensor(
                    comb[:, :, 0 : 2 * C], v[:, :, :, 0, :], v[:, :, :, 1, :], op=vmax
                )
                nc.vector.tensor_tensor(
                    comb[:, :, 2 * C : 4 * C], v[:, :, :, 0, :], v[:, :, :, 1, :], op=vmin
                )

                # transpose chunkA per group: [128e,128f] -> [128f,128e]; 8/bank
                ca_tiles = []
                for q in range(G // 8):
                    pa_t = pap.tile([128, 1024], FP16)
                    for j in range(8):
                        g = 8 * q + j
                        nc.tensor.transpose(
                            pa_t[:, 128 * j : 128 * (j + 1)], comb[:, g, :], ident[:]
                        )
                    ca = ctap.tile([128, 1024], FP16)
                    # PSUM->SBUF evacuation split ~1.5 DVE / ~2.5 Act so both
                    # engines stay just under the per-block DMA roofline
                    if q == 0:
                        nc.vector.tensor_copy(ca[:], pa_t[:])
                    elif q == 1:
                        nc.vector.tensor_copy(ca[:, 0:512], pa_t[:, 0:512])
                        nc.scalar.copy(ca[:, 512:1024], pa_t[:, 512:1024])
                    else:
                        nc.scalar.copy(ca[:], pa_t[:])
                    ca_tiles.append(ca)

                out_sb = osp.tile([128, G, OUT], FP16)
                for ob in range(G // 8):
                    po_t = pop.tile([128, 8, OUT], F32)
                    for k in range(8):
                        g = 8 * ob + k
                        og = po_t[:, k, :]
                        nc.tensor.matmul(
                            og,
                            lhsT=ca_tiles[g // 8][:, 128 * (g % 8) : 128 * (g % 8 + 1)],
                            rhs=wa_sb[:],
                            start=True,
                            stop=False,
                            skip_group_check=True,
                        )
                        nc.tensor.matmul(
                            og,
                            lhsT=xf_t[:, g, :],
                            rhs=wx_sb[:],
                            start=False,
                            stop=True,
                            skip_group_check=True,
                        )
                    nc.scalar.copy(out_sb[:, 8 * ob : 8 * (ob + 1), :], po_t[:])

                # issue the output DMA from the Act sequencer: on SP its
                # sem-wait on out_sb would stall the next block's input DMAs,
                # and on Pool the SWDGE path burns ~1us of Pool engine time.
                # Act wrote out_sb itself, so its in-order SEQ reaches this
                # DMA with the wait already satisfied.
                nc.scalar.dma_start(
                    out[b * EPB : (b + 1) * EPB].rearrange("(p g) d -> p g d", p=128),
                    out_sb[:],
                )

    nc.compile()
    return nc


_NC = None


def _get_nc():
    global _NC
    if _NC is None:
        _NC = _build()
    return _NC


def _host_prep(x, neighbors, W, b):
    x = np.ascontiguousarray(np.asarray(x, dtype=np.float32))
    neighbors = np.asarray(neighbors)
    W = np.asarray(W, dtype=np.float64)
    b = np.asarray(b, dtype=np.float64)
    assert x.shape == (E, C) and neighbors.shape == (E, 4)

    xg = np.concatenate([x, np.zeros((1, C), np.float32)], axis=0).astype(np.float16)

    nb_pad = np.full((E_PAD, 4), E, dtype=np.int64)
    nb_pad[: neighbors.shape[0]] = neighbors
    nb_pad = np.where(nb_pad < 0, E, nb_pad)
    nb_pad = nb_pad[:, [0, 2, 1, 3]]            # per edge: [a0, b0, a1, b1]
    xs_pad = np.zeros((E_PAD, C), np.float16)
    xs_pad[: x.shape[0]] = x

    # W = [W1|W2|W3|W4|W5] along the 5C input features; fold abs-of-difference
    # pairs into sum/difference weights applied to (max, min) features.
    W1, W2, W3, W4, W5 = (W[:, i * C : (i + 1) * C] for i in range(5))
    # rows ordered [MP, MX, mP, MN] to match the device comb layout
    wa = np.concatenate(
        [(W2 + W4).T, (W3 + W5).T, (W2 - W4).T, (W3 - W5).T], axis=0
    ).astype(np.float16)
    wx = np.concatenate([W1.T, b[None, :]], axis=0).astype(np.float16)

    in_maps = []
    for c in range(NCORES):
        lo, hi = c * SHARD, (c + 1) * SHARD
        # edge (blk, p, g) = lo + blk*EPB + 32p + g
        nbd = xg[nb_pad[lo:hi].ravel()].reshape(NBLK, 128, G * 4 * C)
        xfm = xs_pad[lo:hi].reshape(NBLK, 128, G, C).transpose(3, 0, 2, 1)
        xfm = np.concatenate(
            [xfm, np.ones((1, NBLK, G, 128), np.float16)], axis=0
        ).reshape(C + 1, NBLK, G * 128)
        in_maps.append(
            {
                "nbd": np.ascontiguousarray(nbd),
                "xfm": np.ascontiguousarray(xfm),
                "wa": wa,
                "wx": wx,
            }
        )

    return in_maps


def kernel(x, neighbors, W, b):
    n_edges = np.asarray(neighbors).shape[0]
    nc = _get_nc()
    in_maps = _host_prep(x, neighbors, W, b)
    res = run_bass_kernel_spmd(nc, in_maps, core_ids=list(range(NCORES)))
    outs = [r["out"] for r in res.results]
    return np.concatenate(outs, axis=0)[:n_edges].astype(np.float32)
